# revision 1
# baseline (speedup 1.0000x reference)
"""Causal self-attention Trainium2 kernel.

Full input qkv (B=2, S=4096, 3, H=16, D=64) fp32 -> out (B, S, H, D) fp32.
Sharded over 8 cores by (batch, head): core c handles b = c // 4 and the
4 heads h in [(c % 4) * 4, (c % 4) * 4 + 4).

Per core, heads are processed in pairs (A, B). Layout per pair:
  qT2/kT2 [128, S] f16 SBUF: partitions 0-63 = head A's d-dim (q pre-scaled
  by D**-0.5), partitions 64-127 = head B's.
  v [128, S/128, 65] f16 per head: col 64 = 1.0 (ones column -> row sums).
For each s-block (512 q positions) and causal t-block (128 kv positions):
  scoresT[t, s] = kT.T @ qT via two row-packed matmuls (K=64 each,
  tile_position (0,0)/(64,0)) into one [128, 1024] PSUM tile (both heads),
  one exp on ACT -> f16 SBUF (s-sliced on diagonal blocks), causal masking
  of diagonal blocks via gpsimd.affine_select, then out^T[65, s] +=
  v_ext.T @ expT accumulated in PSUM; row 64 accumulates the softmax
  denominator. At s-block end: transpose back on PE, one reciprocal + one
  broadcast multiply, stage per-head output, single DMA per head.
"""

import numpy as np
from contextlib import ExitStack

B, S, H, D = 2, 4096, 16, 64
N_CORES = 8
SB = 512  # s-block width (q positions per block)
TB = 128  # t-block width (kv positions per block)
SCALE = float(D) ** -0.5

_cache = {}


def _build(seq_len, n_heads, repeat=1):
    import concourse.bass as bass
    import concourse.mybir as mybir
    import concourse.tile as tile
    from concourse import bacc
    from concourse.bass import ts
    from concourse.masks import make_identity

    f32 = mybir.dt.float32
    f16 = mybir.dt.float16
    EXP = mybir.ActivationFunctionType.Exp

    NSB = seq_len // SB  # s-blocks / groups
    NCH = seq_len // TB  # 128-chunks
    CPG = SB // TB  # chunks per group (4)
    n_pairs = n_heads // 2

    nc = bacc.Bacc("TRN2", target_bir_lowering=False, debug=False, num_devices=8)
    qkv_t = nc.dram_tensor("qkv", [seq_len, 3, n_heads, D], f32, kind="ExternalInput")
    out_t = nc.dram_tensor("out", [seq_len, n_heads, D], f32, kind="ExternalOutput")
    qkv = qkv_t.ap()
    out = out_t.ap()

    with ExitStack() as ctx:
        tc = ctx.enter_context(tile.TileContext(nc))
        const_pool = ctx.enter_context(tc.tile_pool(name="const", bufs=1))
        stage = ctx.enter_context(tc.tile_pool(name="stage", bufs=3))
        cvt = ctx.enter_context(tc.tile_pool(name="cvt", bufs=4))
        big = ctx.enter_context(tc.tile_pool(name="big", bufs=1))
        ost = ctx.enter_context(tc.tile_pool(name="ost", bufs=1))
        pt_pool = ctx.enter_context(tc.tile_pool(name="pt", bufs=4))
        norm_pool = ctx.enter_context(tc.tile_pool(name="norm", bufs=3))
        ps_pool = ctx.enter_context(tc.tile_pool(name="ps", bufs=2, space="PSUM"))
        po_pool = ctx.enter_context(tc.tile_pool(name="po", bufs=1, space="PSUM"))
        sm_pool = ctx.enter_context(tc.tile_pool(name="sm", bufs=2, space="PSUM"))

        ident16 = const_pool.tile([128, 128], f16)
        make_identity(nc, ident16[:])
        ident32 = const_pool.tile([128, 128], f32)
        make_identity(nc, ident32[:])

        pair_tiles = {}
        out_all = ost.tile([128, NCH, n_heads, D], f32, name="out_all")
        nc.vector.memset(out_all[:], 0.0)

        def make_pair_tiles(p):
            qT2 = big.tile([128, seq_len], f16, tag=f"qT2_{p % 2}", name=f"qT2_{p}")
            kT2 = big.tile([128, seq_len], f16, tag=f"kT2_{p % 2}", name=f"kT2_{p}")
            v0 = big.tile([128, NCH, D + 1], f16, tag=f"v0_{p % 2}", name=f"v0_{p}")
            v1 = big.tile([128, NCH, D + 1], f16, tag=f"v1_{p % 2}", name=f"v1_{p}")
            pair_tiles[p] = (qT2, kT2, v0, v1)

        prep_bufs = {}

        def prep_load(c):
            """Load chunk c (128 seq positions) of q, k, v for ALL heads with
            one DMA ([128, 3, n_heads, D]) and convert to f16."""
            st = stage.tile([TB, 3, n_heads, D], f32, tag="st", name=f"st_{c}")
            nc.sync.dma_start(st[:], qkv[c * TB : (c + 1) * TB, :, :, :])
            bufs = []
            for p in range(n_pairs):
                _, _, v0, v1 = pair_tiles[p]
                for e, vt in enumerate([v0, v1]):
                    h = 2 * p + e
                    nc.vector.tensor_copy(vt[:, c, 0:D], st[:, 2, h, :])
                    nc.gpsimd.memset(vt[:, c, D : D + 1], 1.0)
                    cq = cvt.tile([TB, D], f16, tag=f"cq{p}_{e}", name=f"cq{p}_{e}_{c}")
                    nc.vector.tensor_scalar_mul(cq[:], st[:, 0, h, :], SCALE)
                    ck = cvt.tile([TB, D], f16, tag=f"ck{p}_{e}", name=f"ck{p}_{e}_{c}")
                    nc.vector.tensor_copy(ck[:], st[:, 1, h, :])
                    bufs.append((p, e, cq, ck))
            prep_bufs[c] = bufs

        def prep_store(c):
            """PE-transpose chunk c into qT2/kT2 [d, s] layout."""
            sl = slice(c * TB, (c + 1) * TB)
            pq = sm_pool.tile([128, 2 * n_pairs, TB], f16, tag="small", name=f"pq_{c}")
            for p, e, cq, ck in prep_bufs[c]:
                pr, sr = (0, D) if e == 0 else (D, 128)
                tp = None if e == 0 else (0, 64)
                nc.tensor.transpose(pq[pr:sr, 2 * p, :], cq[:], ident16[:], tile_position=tp)
                nc.tensor.transpose(
                    pq[pr:sr, 2 * p + 1, :], ck[:], ident16[:], tile_position=tp
                )
            for p in range(n_pairs):
                qT2, kT2, _, _ = pair_tiles[p]
                nc.vector.tensor_copy(qT2[:, sl], pq[:, 2 * p, :])
                nc.vector.tensor_copy(kT2[:, sl], pq[:, 2 * p + 1, :])
            del prep_bufs[c]

        def prep_chunk(c):
            prep_load(c)
            prep_store(c)

        out_tiles = {}

        SLICE_DIAG = True

        def emit_qk(p, i, j):
            qT2, kT2, _, _ = pair_tiles[p]
            off = TB * (j - CPG * i) if (SLICE_DIAG and j > CPG * i) else 0
            ps = ps_pool.tile([128, 2, SB], f32, tag="ps", name=f"ps_{p}_{i}_{j}")
            tsl = slice(j * TB, (j + 1) * TB)
            nc.tensor.matmul(
                ps[:, 0, off:SB],
                kT2[0:D, tsl],
                qT2[0:D, i * SB + off : (i + 1) * SB],
                start=True,
                stop=True,
            )
            nc.tensor.matmul(
                ps[:, 1, off:SB],
                kT2[D:128, tsl],
                qT2[D:128, i * SB + off : (i + 1) * SB],
                start=True,
                stop=True,
                tile_position=(64, 0),
            )
            return ps

        def emit_expmask(p, i, j, ps_cur):
            m = j - CPG * i
            off = TB * m if (SLICE_DIAG and m > 0) else 0
            pt = pt_pool.tile([128, 2, SB], f16, tag="pt", name=f"pt_{p}_{i}_{j}")
            if off:
                nc.gpsimd.memset(pt[:, :, 0:off], 0.0)
            nc.scalar.activation(pt[:, :, off:SB], ps_cur[:, :, off:SB], EXP)
            if m >= 0:
                # masked-out region ends at s' = TB*m + t < TB*(m+1)
                hi = TB * (m + 1)
                for half in range(2):
                    sub = pt[:, half, off:hi]
                    nc.gpsimd.affine_select(
                        out=sub,
                        in_=sub,
                        compare_op=mybir.AluOpType.is_ge,
                        fill=0.0,
                        base=off - TB * m,
                        channel_multiplier=-1,
                        pattern=[[1, hi - off]],
                    )
            return pt

        def emit_av(p, i, j, pt):
            _, _, v0, v1 = pair_tiles[p]
            nt = CPG * (i + 1)
            if j == 0:
                outA = po_pool.tile([D + 1, SB], f32, tag="oA", name=f"oA_{p}_{i}")
                outB = po_pool.tile([D + 1, SB], f32, tag="oB", name=f"oB_{p}_{i}")
                out_tiles[(p, i)] = (outA, outB)
            outA, outB = out_tiles[(p, i)]
            nc.tensor.matmul(
                outA[:], v0[:, j, :], pt[:, 0, :], start=(j == 0), stop=(j == nt - 1)
            )
            nc.tensor.matmul(
                outB[:], v1[:, j, :], pt[:, 1, :], start=(j == 0), stop=(j == nt - 1)
            )

        norm_bufs = {}

        def emit_norm_copy(p, i, e):
            o = out_tiles[(p, i)][e]
            onr = norm_pool.tile([D + 1, SB], f32, tag="onr", name=f"onr_{p}_{i}_{e}")
            nc.vector.tensor_copy(onr[:], o[:])
            norm_bufs[(p, i, e)] = onr

        def emit_norm(p, i, e):
            h = 2 * p + e
            onr = norm_bufs.pop((p, i, e))
            tp4 = sm_pool.tile([128, CPG, D + 1], f32, tag="small", name=f"tp4_{p}_{i}_{e}")
            for c4 in range(CPG):
                nc.tensor.transpose(
                    tp4[:, c4, :], onr[:, ts(c4, TB)], ident32[0 : D + 1, 0 : D + 1]
                )
            rc4 = norm_pool.tile([128, CPG, 1], f32, tag="rc4", name=f"rc4_{p}_{i}_{e}")
            nc.vector.reciprocal(rc4[:], tp4[:, :, D : D + 1])
            for c4 in range(CPG):
                nc.vector.tensor_scalar_mul(
                    out_all[:, i * CPG + c4, h, :], tp4[:, c4, 0:D], rc4[:, c4, :]
                )

        def emit_outdma(i):
            dst = out[i * SB : (i + 1) * SB, :, :].rearrange(
                "(a pp) h d -> pp a h d", pp=TB
            )
            nc.gpsimd.dma_start(dst, out_all[:, i * CPG : (i + 1) * CPG, :, :])

        import functools

        for rep in range(repeat):
            if rep == 0:
                for p in range(n_pairs):
                    make_pair_tiles(p)
            units = [
                (p, i, j)
                for p in range(n_pairs)
                for i in range(NSB)
                for j in range(CPG * (i + 1))
            ]
            extras = {k: [] for k in range(len(units))}
            tail = []
            base_of = {}
            for k, (p, i, j) in enumerate(units):
                if j == 0:
                    base_of[(p, i)] = k

            def attach(p, i, tasks):
                nxt = (p, i + 1) if i + 1 < NSB else (p + 1, 0)
                if nxt not in base_of:
                    tail.extend(tasks)
                    return
                base = base_of[nxt]
                nu = CPG * (nxt[1] + 1)
                nt_ = len(tasks)
                # cap at unit nu-3: the next block's first qk is emitted (via
                # lookahead) during unit nu-1, and unit nu-2's extras follow
                # that block's own last-unit lookahead; staying two units
                # clear keeps every prep write emitted before its readers.
                cap = max(nu - 3, 0)
                for t_idx, task in enumerate(tasks):
                    k = base + 1 + min(t_idx * max(nu - 1, 1) // nt_, cap)
                    extras[k].append(task)

            for p in range(n_pairs):
                for i in range(NSB):
                    tasks = [
                        functools.partial(emit_norm, p, i, 0),
                        functools.partial(emit_norm, p, i, 1),
                    ]
                    if p == n_pairs - 1:
                        tasks.append(functools.partial(emit_outdma, i))
                    if p == 0:
                        # prep runs during s-block i+1; its chunks must be
                        # ready before s-block i+2's first qk is EMITTED, so
                        # prep chunks for block i+2 here (0..7 done upfront)
                        for c in range(CPG * (i + 2), min(CPG * (i + 3), NCH)):
                            tasks.append(functools.partial(prep_load, c))
                            tasks.append(functools.partial(prep_store, c))
                    attach(p, i, tasks)

            for c in range(min(2 * CPG, NCH)):
                prep_chunk(c)
            ps_cur = emit_qk(*units[0])
            for k, u in enumerate(units):
                pt = emit_expmask(*u, ps_cur)
                ps_cur = emit_qk(*units[k + 1]) if k + 1 < len(units) else None
                emit_av(*u, pt)
                p_, i_, j_ = u
                if j_ == CPG * (i_ + 1) - 1:
                    emit_norm_copy(p_, i_, 0)
                    emit_norm_copy(p_, i_, 1)
                for task in extras[k]:
                    task()
            for task in tail:
                task()

    nc.compile()
    return nc


def get_nc(seq_len=S, n_heads=H * B // N_CORES, repeat=1):
    key = (seq_len, n_heads, repeat)
    if key not in _cache:
        _cache[key] = _build(seq_len, n_heads, repeat)
    return _cache[key]


def kernel(qkv: np.ndarray) -> np.ndarray:
    from concourse.bass_utils import run_bass_kernel_spmd

    qkv = np.ascontiguousarray(np.asarray(qkv, dtype=np.float32))
    assert qkv.shape == (B, S, 3, H, D)
    hpc = H * B // N_CORES  # heads per core
    cores_per_b = H // hpc
    ins = []
    for c in range(N_CORES):
        b, h0 = c // cores_per_b, (c % cores_per_b) * hpc
        ins.append({"qkv": np.ascontiguousarray(qkv[b, :, :, h0 : h0 + hpc, :])})
    nc = get_nc()
    res = run_bass_kernel_spmd(nc, ins, core_ids=list(range(N_CORES)))
    full = np.empty((B, S, H, D), np.float32)
    for c in range(N_CORES):
        b, h0 = c // cores_per_b, (c % cores_per_b) * hpc
        full[b, :, h0 : h0 + hpc, :] = res.results[c]["out"]
    return full



# revision 2
# speedup vs baseline: 1.0027x; 1.0027x over previous
"""Causal self-attention Trainium2 kernel.

Full input qkv (B=2, S=4096, 3, H=16, D=64) fp32 -> out (B, S, H, D) fp32.
Sharded over 8 cores by (batch, head): core c handles b = c // 4 and the
4 heads h in [(c % 4) * 4, (c % 4) * 4 + 4).

Per core, heads are processed in pairs (A, B). Layout per pair:
  qT2/kT2 [128, S] f16 SBUF: partitions 0-63 = head A's d-dim (q pre-scaled
  by D**-0.5), partitions 64-127 = head B's.
  v4 [128, NCH, n_heads, 65] f16 shared: col 64 = 1.0 (ones -> row sums).
For each s-block (512 q positions) and causal t-block (128 kv positions):
  scoresT[t, s] = kT.T @ qT via two row-packed matmuls (K=64 each,
  tile_position (0,0)/(64,0)) into one [128, 1024] PSUM tile (both heads);
  diagonal blocks get causal masking via one extra PE matmul accumulating a
  strictly-lower-triangular -30000 tile (identity weights) into the psum.
  exp runs on ACT (exact) or, for a tunable share of off-diagonal blocks,
  on DVE via a Schraudolph fast-exp (scores * 1024/ln2 + bias -> int16
  round-to-nearest-saturate -> bitcast f16); then out^T[65, s] +=
  v_ext.T @ expT accumulated in PSUM (sliced to the causal column range);
  row 64 accumulates the softmax denominator. At s-block end: transpose
  back on PE, one reciprocal + one broadcast multiply, stage per-head
  output, single DMA per s-block.

Prep (per 128-seq chunk): one DMA of [128, 3, H, D] f32, three batched
gpsimd converts (q*scale, k, v -> f16), then 2-head-packed PE transposes
into the [d, s] layout.
"""

import numpy as np
from contextlib import ExitStack

B, S, H, D = 2, 4096, 16, 64
N_CORES = 8
SB = 512  # s-block width (q positions per block)
TB = 128  # t-block width (kv positions per block)
SCALE = float(D) ** -0.5
MASK_VAL = -30000.0

# Schraudolph fast-exp on DVE: f16 bits = rne(x * 1024/ln2 + (15*1024 - 45))
SH_A = 1024.0 / float(np.log(2.0))
SH_B = 15.0 * 1024.0 - 45.0
# fraction control: off-diagonal units with (hash % DVE_DEN) < DVE_NUM go to DVE
DVE_NUM, DVE_DEN = 9, 20

_cache = {}


def _build(seq_len, n_heads, repeat=1):
    import concourse.bass as bass
    import concourse.mybir as mybir
    import concourse.tile as tile
    from concourse import bacc
    from concourse.bass import ts
    from concourse.masks import make_identity

    f32 = mybir.dt.float32
    f16 = mybir.dt.float16
    i16 = mybir.dt.int16
    EXP = mybir.ActivationFunctionType.Exp

    NSB = seq_len // SB  # s-blocks / groups
    NCH = seq_len // TB  # 128-chunks
    CPG = SB // TB  # chunks per group (4)
    n_pairs = n_heads // 2

    nc = bacc.Bacc("TRN2", target_bir_lowering=False, debug=False, num_devices=8)
    qkv_t = nc.dram_tensor("qkv", [seq_len, 3, n_heads, D], f32, kind="ExternalInput")
    out_t = nc.dram_tensor("out", [seq_len, n_heads, D], f32, kind="ExternalOutput")
    qkv = qkv_t.ap()
    out = out_t.ap()

    with ExitStack() as ctx:
        tc = ctx.enter_context(tile.TileContext(nc))
        const_pool = ctx.enter_context(tc.tile_pool(name="const", bufs=1))
        stage = ctx.enter_context(tc.tile_pool(name="stage", bufs=3))
        cvt = ctx.enter_context(tc.tile_pool(name="cvt", bufs=4))
        big = ctx.enter_context(tc.tile_pool(name="big", bufs=1))
        ost = ctx.enter_context(tc.tile_pool(name="ost", bufs=1))
        pt_pool = ctx.enter_context(tc.tile_pool(name="pt", bufs=4))
        norm_pool = ctx.enter_context(tc.tile_pool(name="norm", bufs=3))
        ps_pool = ctx.enter_context(tc.tile_pool(name="ps", bufs=2, space="PSUM"))
        po_pool = ctx.enter_context(tc.tile_pool(name="po", bufs=1, space="PSUM"))
        sm_pool = ctx.enter_context(tc.tile_pool(name="sm", bufs=2, space="PSUM"))

        ident16 = const_pool.tile([128, 128], f16)
        make_identity(nc, ident16[:])
        ident32 = const_pool.tile([128, 128], f32)
        make_identity(nc, ident32[:])

        # Strictly-lower-triangular causal mask tile, duplicated for the two
        # heads of a pair: maskT[t, e, u] = MASK_VAL if u < t else 0.
        maskT = const_pool.tile([128, 2, 128], f16)
        nc.vector.memset(maskT[:], 0.0)
        nc.gpsimd.affine_select(
            out=maskT[:],
            in_=maskT[:],
            compare_op=mybir.AluOpType.is_ge,
            fill=MASK_VAL,
            base=0,
            channel_multiplier=-1,
            pattern=[[0, 2], [1, 128]],
        )

        pair_tiles = {}
        out_all = ost.tile([128, NCH, n_heads, D], f32, name="out_all")
        # v4: all heads' v in f16, with a ones column at index D for row sums.
        v4 = big.tile([128, NCH, n_heads, D + 1], f16, name="v4")
        nc.vector.memset(v4[:, :, :, D : D + 1], 1.0)

        def make_pair_tiles(p):
            qT2 = big.tile([128, seq_len], f16, tag=f"qT2_{p % 2}", name=f"qT2_{p}")
            kT2 = big.tile([128, seq_len], f16, tag=f"kT2_{p % 2}", name=f"kT2_{p}")
            pair_tiles[p] = (qT2, kT2)

        prep_bufs = {}

        def prep_load(c):
            """Load chunk c (128 seq positions) of q, k, v for ALL heads with
            one DMA ([128, 3, n_heads, D]) and convert to f16 on gpsimd."""
            st = stage.tile([TB, 3, n_heads, D], f32, tag="st", name=f"st_{c}")
            nc.sync.dma_start(st[:], qkv[c * TB : (c + 1) * TB, :, :, :])
            nc.gpsimd.tensor_copy(v4[:, c, :, 0:D], st[:, 2, :, :])
            cqk = cvt.tile([TB, 2, n_heads, D], f16, tag="cqk", name=f"cqk_{c}")
            nc.gpsimd.tensor_scalar_mul(cqk[:, 0, :, :], st[:, 0, :, :], SCALE)
            nc.gpsimd.tensor_copy(cqk[:, 1, :, :], st[:, 1, :, :])
            prep_bufs[c] = cqk

        def prep_store(c):
            """PE-transpose chunk c into qT2/kT2 [d, s] layout, two heads per
            transpose (the packed [128, 2*64] input lands as partitions 0-63 =
            head A's d, 64-127 = head B's)."""
            sl = slice(c * TB, (c + 1) * TB)
            cqk = prep_bufs.pop(c)
            pq = sm_pool.tile([128, 2 * n_pairs, TB], f16, tag="small", name=f"pq_{c}")
            for p in range(n_pairs):
                nc.tensor.transpose(
                    pq[:, 2 * p, :], cqk[:, 0, 2 * p : 2 * p + 2, :], ident16[:]
                )
                nc.tensor.transpose(
                    pq[:, 2 * p + 1, :], cqk[:, 1, 2 * p : 2 * p + 2, :], ident16[:]
                )
            for p in range(n_pairs):
                qT2, kT2 = pair_tiles[p]
                nc.vector.tensor_copy(qT2[:, sl], pq[:, 2 * p, :])
                nc.vector.tensor_copy(kT2[:, sl], pq[:, 2 * p + 1, :])

        def prep_chunk(c):
            prep_load(c)
            prep_store(c)

        out_tiles = {}

        def emit_qk(p, i, j):
            qT2, kT2 = pair_tiles[p]
            m = j - CPG * i
            off = TB * m if m > 0 else 0
            ps = ps_pool.tile([128, 2, SB], f32, tag="ps", name=f"ps_{p}_{i}_{j}")
            tsl = slice(j * TB, (j + 1) * TB)
            nc.tensor.matmul(
                ps[:, 0, off:SB],
                kT2[0:D, tsl],
                qT2[0:D, i * SB + off : (i + 1) * SB],
                start=True,
                stop=(m < 0),
            )
            nc.tensor.matmul(
                ps[:, 1, off:SB],
                kT2[D:128, tsl],
                qT2[D:128, i * SB + off : (i + 1) * SB],
                start=True,
                stop=(m < 0),
                tile_position=(64, 0),
            )
            if m >= 0:
                # causal masking of the diagonal 128-col band via PE:
                # ps[:, :, TB*m : TB*(m+1)] += I.T @ maskT
                nc.tensor.matmul(
                    ps[:, :, TB * m : TB * (m + 1)],
                    ident16[:],
                    maskT[:],
                    start=False,
                    stop=True,
                    skip_group_check=True,
                )
            return ps

        def emit_exp(p, i, j, ps_cur):
            m = j - CPG * i
            off = TB * m if m > 0 else 0
            pt = pt_pool.tile([128, 2, SB], f16, tag="pt", name=f"pt_{p}_{i}_{j}")
            use_dve = m < 0 and i >= 1 and ((i * CPG + j + p) % DVE_DEN) < DVE_NUM
            if use_dve:
                nc.vector.tensor_scalar(
                    pt[:, :, :].bitcast(i16),
                    ps_cur[:, :, :],
                    SH_A,
                    SH_B,
                    op0=mybir.AluOpType.mult,
                    op1=mybir.AluOpType.add,
                )
            else:
                nc.scalar.activation(pt[:, :, off:SB], ps_cur[:, :, off:SB], EXP)
            return pt

        def emit_av(p, i, j, pt):
            m = j - CPG * i
            off = TB * m if m > 0 else 0
            nt = CPG * (i + 1)
            if j == 0:
                outA = po_pool.tile([D + 1, SB], f32, tag="oA", name=f"oA_{p}_{i}")
                outB = po_pool.tile([D + 1, SB], f32, tag="oB", name=f"oB_{p}_{i}")
                out_tiles[(p, i)] = (outA, outB)
            outA, outB = out_tiles[(p, i)]
            for e, o in enumerate([outA, outB]):
                nc.tensor.matmul(
                    o[:, off:SB],
                    v4[:, j, 2 * p + e, :],
                    pt[:, e, off:SB],
                    start=(j == 0),
                    stop=(j == nt - 1),
                    skip_group_check=True,
                )

        norm_bufs = {}

        def emit_norm_copy(p, i, e):
            o = out_tiles[(p, i)][e]
            onr = norm_pool.tile([D + 1, SB], f32, tag="onr", name=f"onr_{p}_{i}_{e}")
            nc.vector.tensor_copy(onr[:], o[:])
            norm_bufs[(p, i, e)] = onr

        def emit_norm(p, i, e):
            h = 2 * p + e
            onr = norm_bufs.pop((p, i, e))
            tp4 = sm_pool.tile([128, CPG, D + 1], f32, tag="small", name=f"tp4_{p}_{i}_{e}")
            for c4 in range(CPG):
                nc.tensor.transpose(
                    tp4[:, c4, :], onr[:, ts(c4, TB)], ident32[0 : D + 1, 0 : D + 1]
                )
            rc4 = norm_pool.tile([128, CPG, 1], f32, tag="rc4", name=f"rc4_{p}_{i}_{e}")
            nc.vector.reciprocal(rc4[:], tp4[:, :, D : D + 1])
            for c4 in range(CPG):
                nc.vector.tensor_scalar_mul(
                    out_all[:, i * CPG + c4, h, :], tp4[:, c4, 0:D], rc4[:, c4, :]
                )

        def emit_outdma(i):
            dst = out[i * SB : (i + 1) * SB, :, :].rearrange(
                "(a pp) h d -> pp a h d", pp=TB
            )
            nc.gpsimd.dma_start(dst, out_all[:, i * CPG : (i + 1) * CPG, :, :])

        import functools

        for rep in range(repeat):
            if rep == 0:
                for p in range(n_pairs):
                    make_pair_tiles(p)
            units = [
                (p, i, j)
                for p in range(n_pairs)
                for i in range(NSB)
                for j in range(CPG * (i + 1))
            ]
            extras = {k: [] for k in range(len(units))}
            tail = []
            base_of = {}
            for k, (p, i, j) in enumerate(units):
                if j == 0:
                    base_of[(p, i)] = k

            def attach(p, i, tasks):
                nxt = (p, i + 1) if i + 1 < NSB else (p + 1, 0)
                if nxt not in base_of:
                    tail.extend(tasks)
                    return
                base = base_of[nxt]
                nu = CPG * (nxt[1] + 1)
                nt_ = len(tasks)
                # cap at unit nu-3: the next block's first qk is emitted (via
                # lookahead) during unit nu-1, and unit nu-2's extras follow
                # that block's own last-unit lookahead; staying two units
                # clear keeps every prep write emitted before its readers.
                cap = max(nu - 3, 0)
                for t_idx, task in enumerate(tasks):
                    k = base + 1 + min(t_idx * max(nu - 1, 1) // nt_, cap)
                    extras[k].append(task)

            for p in range(n_pairs):
                for i in range(NSB):
                    tasks = [
                        functools.partial(emit_norm, p, i, 0),
                        functools.partial(emit_norm, p, i, 1),
                    ]
                    if p == n_pairs - 1:
                        tasks.append(functools.partial(emit_outdma, i))
                    if p == 0:
                        # prep runs during s-block i+1; its chunks must be
                        # ready before s-block i+2's first qk is EMITTED, so
                        # prep chunks for block i+2 here (0..7 done upfront)
                        for c in range(CPG * (i + 2), min(CPG * (i + 3), NCH)):
                            tasks.append(functools.partial(prep_load, c))
                            tasks.append(functools.partial(prep_store, c))
                    attach(p, i, tasks)

            for c in range(min(2 * CPG, NCH)):
                prep_chunk(c)
            ps_cur = emit_qk(*units[0])
            for k, u in enumerate(units):
                pt = emit_exp(*u, ps_cur)
                ps_cur = emit_qk(*units[k + 1]) if k + 1 < len(units) else None
                emit_av(*u, pt)
                p_, i_, j_ = u
                if j_ == CPG * (i_ + 1) - 1:
                    emit_norm_copy(p_, i_, 0)
                    emit_norm_copy(p_, i_, 1)
                for task in extras[k]:
                    task()
            for task in tail:
                task()

    nc.compile()
    return nc


def get_nc(seq_len=S, n_heads=H * B // N_CORES, repeat=1):
    key = (seq_len, n_heads, repeat)
    if key not in _cache:
        _cache[key] = _build(seq_len, n_heads, repeat)
    return _cache[key]


def kernel(qkv: np.ndarray) -> np.ndarray:
    from concourse.bass_utils import run_bass_kernel_spmd

    qkv = np.ascontiguousarray(np.asarray(qkv, dtype=np.float32))
    assert qkv.shape == (B, S, 3, H, D)
    hpc = H * B // N_CORES  # heads per core
    cores_per_b = H // hpc
    ins = []
    for c in range(N_CORES):
        b, h0 = c // cores_per_b, (c % cores_per_b) * hpc
        ins.append({"qkv": np.ascontiguousarray(qkv[b, :, :, h0 : h0 + hpc, :])})
    nc = get_nc()
    res = run_bass_kernel_spmd(nc, ins, core_ids=list(range(N_CORES)))
    full = np.empty((B, S, H, D), np.float32)
    for c in range(N_CORES):
        b, h0 = c // cores_per_b, (c % cores_per_b) * hpc
        full[b, :, h0 : h0 + hpc, :] = res.results[c]["out"]
    return full


# revision 12
# speedup vs baseline: 1.4081x; 1.4043x over previous
"""Causal self-attention Trainium2 kernel.

Full input qkv (B=2, S=4096, 3, H=16, D=64) fp32 -> out (B, S, H, D) fp32.
Sharded over 8 cores by (batch, head): core c handles b = c // 4 and the
4 heads h in [(c % 4) * 4, (c % 4) * 4 + 4).

Per core, heads are processed in pairs (A, B). Layout per pair:
  qT2/kT2 [128, S] f16 SBUF: partitions 0-63 = head A's d-dim (q pre-scaled
  by D**-0.5), partitions 64-127 = head B's.
  v4 [128, NCH, n_heads, 65] f16 shared: col 64 = 1.0 (ones -> row sums).
For each s-block (512 q positions) and causal t-block (128 kv positions):
  scoresT[t, s] = kT.T @ qT via two row-packed matmuls (K=64 each,
  tile_position (0,0)/(64,0)) into one [128, 1024] PSUM tile (both heads);
  diagonal blocks get causal masking via one extra PE matmul accumulating a
  strictly-lower-triangular -30000 tile (identity weights) into the psum.
  exp runs on ACT (exact) or, for a tunable share of off-diagonal blocks,
  on DVE via a Schraudolph fast-exp (scores * 1024/ln2 + bias -> int16
  round-to-nearest-saturate -> bitcast f16); then out^T[65, s] +=
  v_ext.T @ expT accumulated in PSUM (sliced to the causal column range);
  row 64 accumulates the softmax denominator. At s-block end: transpose
  back on PE, one reciprocal + one broadcast multiply, stage per-head
  output, single DMA per s-block.

Prep (per 128-seq chunk): one DMA of [128, 3, H, D] f32, three batched
gpsimd converts (q*scale, k, v -> f16), then 2-head-packed PE transposes
into the [d, s] layout.
"""

import numpy as np
from contextlib import ExitStack

B, S, H, D = 2, 4096, 16, 64
N_CORES = 8
SB = 512  # s-block width (q positions per block)
TB = 128  # t-block width (kv positions per block)
SCALE = float(D) ** -0.5
MASK_VAL = -30000.0

# Schraudolph fast-exp on DVE: f16 bits = rne(x * 1024/ln2 + (15*1024 - 45))
SH_A = 1024.0 / float(np.log(2.0))
SH_B = 15.0 * 1024.0 - 45.0

_cache = {}


def _build(seq_len, n_heads, repeat=1):
    import concourse.bass as bass
    import concourse.mybir as mybir
    import concourse.tile as tile
    from concourse import bacc
    from concourse.bass import ts
    from concourse.masks import make_identity

    f32 = mybir.dt.float32
    f16 = mybir.dt.float16
    i16 = mybir.dt.int16
    EXP = mybir.ActivationFunctionType.Exp

    NSB = seq_len // SB  # s-blocks / groups
    NCH = seq_len // TB  # 128-chunks
    CPG = SB // TB  # chunks per group (4)
    n_pairs = n_heads // 2

    nc = bacc.Bacc("TRN2", target_bir_lowering=False, debug=False, num_devices=8)
    qkv_t = nc.dram_tensor("qkv", [seq_len, 3, n_heads, D], f32, kind="ExternalInput")
    out_t = nc.dram_tensor("out", [seq_len, n_heads, D], f32, kind="ExternalOutput")
    qkv = qkv_t.ap()
    out = out_t.ap()

    with ExitStack() as ctx:
        tc = ctx.enter_context(tile.TileContext(nc))
        const_pool = ctx.enter_context(tc.tile_pool(name="const", bufs=1))
        stage = ctx.enter_context(tc.tile_pool(name="stage", bufs=3))
        cvt = ctx.enter_context(tc.tile_pool(name="cvt", bufs=4))
        big = ctx.enter_context(tc.tile_pool(name="big", bufs=1))
        ost = ctx.enter_context(tc.tile_pool(name="ost", bufs=1))
        pt_pool = ctx.enter_context(tc.tile_pool(name="pt", bufs=4))
        norm_pool = ctx.enter_context(tc.tile_pool(name="norm", bufs=3))
        ps_pool = ctx.enter_context(tc.tile_pool(name="ps", bufs=2, space="PSUM"))
        po_pool = ctx.enter_context(tc.tile_pool(name="po", bufs=1, space="PSUM"))
        sm_pool = ctx.enter_context(tc.tile_pool(name="sm", bufs=2, space="PSUM"))

        ident16 = const_pool.tile([128, 128], f16)
        make_identity(nc, ident16[:])
        ident32 = const_pool.tile([128, 128], f32)
        make_identity(nc, ident32[:])

        # Strictly-lower-triangular causal mask tile, duplicated for the two
        # heads of a pair: maskT[t, e, u] = MASK_VAL if u < t else 0.
        maskT = const_pool.tile([128, 2, 128], f16)
        nc.vector.memset(maskT[:], 0.0)
        nc.gpsimd.affine_select(
            out=maskT[:],
            in_=maskT[:],
            compare_op=mybir.AluOpType.is_ge,
            fill=MASK_VAL,
            base=0,
            channel_multiplier=-1,
            pattern=[[0, 2], [1, 128]],
        )

        out_all = ost.tile([128, NCH, n_heads, D], f32, name="out_all")
        # v4: all heads' v in f16, with a ones column at index D for row sums.
        v4 = big.tile([128, NCH, n_heads, D + 1], f16, name="v4")
        nc.vector.memset(v4[:, :, :, D : D + 1], 1.0)
        # qk4[:, 2p, :] = head-pair p's q (d-packed, pre-scaled), qk4[:, 2p+1, :]
        # its k; partitions 0-63 = head A's d, 64-127 = head B's.
        qk4 = big.tile([128, 2 * n_pairs, seq_len], f16, name="qk4")

        prep_bufs = {}

        def prep_load(c):
            """Load chunk c (128 seq positions) of q, k, v for ALL heads with
            one DMA ([128, 3, n_heads, D]) and convert to f16 on gpsimd."""
            st = stage.tile([TB, 3, n_heads, D], f32, tag="st", name=f"st_{c}")
            nc.sync.dma_start(st[:], qkv[c * TB : (c + 1) * TB, :, :, :])
            nc.gpsimd.tensor_copy(v4[:, c, :, 0:D], st[:, 2, :, :])
            cqk = cvt.tile([TB, 2, n_heads, D], f16, tag="cqk", name=f"cqk_{c}")
            nc.gpsimd.tensor_scalar_mul(cqk[:, 0, :, :], st[:, 0, :, :], SCALE)
            nc.gpsimd.tensor_copy(cqk[:, 1, :, :], st[:, 1, :, :])
            prep_bufs[c] = cqk

        def prep_store(c):
            """PE-transpose chunk c into the [d, s] layout, two heads per
            transpose (the packed [128, 2*64] input lands as partitions 0-63 =
            head A's d, 64-127 = head B's); one batched copy to qk4."""
            sl = slice(c * TB, (c + 1) * TB)
            cqk = prep_bufs.pop(c)
            pq = sm_pool.tile([128, 2 * n_pairs, TB], f16, tag="small", name=f"pq_{c}")
            for p in range(n_pairs):
                nc.tensor.transpose(
                    pq[:, 2 * p, :], cqk[:, 0, 2 * p : 2 * p + 2, :], ident16[:]
                )
                nc.tensor.transpose(
                    pq[:, 2 * p + 1, :], cqk[:, 1, 2 * p : 2 * p + 2, :], ident16[:]
                )
            nc.vector.tensor_copy(qk4[:, :, sl], pq[:])

        def prep_chunk(c):
            prep_load(c)
            prep_store(c)

        out_tiles = {}

        def emit_qk(p, i, j):
            m = j - CPG * i
            off = TB * m if m > 0 else 0
            psA = ps_pool.tile([128, SB], f32, tag="psA", name=f"psA_{p}_{i}_{j}")
            psB = ps_pool.tile([128, SB], f32, tag="psB", name=f"psB_{p}_{i}_{j}")
            tsl = slice(j * TB, (j + 1) * TB)
            for e, pse, d0, d1 in ((0, psA, 0, D), (1, psB, D, 128)):
                nc.tensor.matmul(
                    pse[:, off:SB],
                    qk4[d0:d1, 2 * p + 1, tsl],
                    qk4[d0:d1, 2 * p, i * SB + off : (i + 1) * SB],
                    start=True,
                    stop=(m < 0),
                    tile_position=(d0, 0),
                )
                if m >= 0:
                    # causal masking of the diagonal 128-col band via PE:
                    # pse[:, TB*m : TB*(m+1)] += I.T @ maskT
                    nc.tensor.matmul(
                        pse[:, TB * m : TB * (m + 1)],
                        ident16[:],
                        maskT[:, e, :],
                        start=False,
                        stop=True,
                        skip_group_check=True,
                    )
            return (psA, psB)

        def emit_exp(p, i, j, ps_cur):
            m = j - CPG * i
            off = TB * m if m > 0 else 0
            psA, psB = ps_cur
            ptA = pt_pool.tile([128, SB], f16, tag="ptA", name=f"ptA_{p}_{i}_{j}")
            ptB = pt_pool.tile([128, SB], f16, tag="ptB", name=f"ptB_{p}_{i}_{j}")
            # head A exact on ACT; head B fast-exp on DVE (concurrent) except
            # for the small-n softmax rows of s-block 0 (kept exact).
            nc.scalar.activation(ptA[:, off:SB], psA[:, off:SB], EXP)
            if i == 0:
                nc.scalar.activation(ptB[:, off:SB], psB[:, off:SB], EXP)
            else:
                nc.vector.tensor_scalar(
                    ptB[:, off:SB].bitcast(i16),
                    psB[:, off:SB],
                    SH_A,
                    SH_B,
                    op0=mybir.AluOpType.mult,
                    op1=mybir.AluOpType.add,
                )
            return (ptA, ptB)

        def emit_av(p, i, j, pt):
            m = j - CPG * i
            off = TB * m if m > 0 else 0
            nt = CPG * (i + 1)
            if j == 0:
                outA = po_pool.tile([D + 1, SB], f32, tag="oA", name=f"oA_{p}_{i}")
                outB = po_pool.tile([D + 1, SB], f32, tag="oB", name=f"oB_{p}_{i}")
                out_tiles[(p, i)] = (outA, outB)
            outA, outB = out_tiles[(p, i)]
            for e, o in enumerate([outA, outB]):
                nc.tensor.matmul(
                    o[:, off:SB],
                    v4[:, j, 2 * p + e, :],
                    pt[e][:, off:SB],
                    start=(j == 0),
                    stop=(j == nt - 1),
                    skip_group_check=True,
                )

        norm_bufs = {}

        def emit_norm_copy(p, i, e):
            o = out_tiles[(p, i)][e]
            onr = norm_pool.tile([D + 1, SB], f32, tag="onr", name=f"onr_{p}_{i}_{e}")
            nc.scalar.copy(onr[:], o[:])
            norm_bufs[(p, i, e)] = onr

        def emit_norm(p, i, e):
            h = 2 * p + e
            onr = norm_bufs.pop((p, i, e))
            tp4 = sm_pool.tile([128, CPG, D + 1], f32, tag="small", name=f"tp4_{p}_{i}_{e}")
            for c4 in range(CPG):
                nc.tensor.transpose(
                    tp4[:, c4, :], onr[:, ts(c4, TB)], ident32[0 : D + 1, 0 : D + 1]
                )
            rc4 = norm_pool.tile([128, CPG, 1], f32, tag="rc4", name=f"rc4_{p}_{i}_{e}")
            nc.vector.reciprocal(rc4[:], tp4[:, :, D : D + 1])
            for c4 in range(CPG):
                nc.vector.tensor_scalar_mul(
                    out_all[:, i * CPG + c4, h, :], tp4[:, c4, 0:D], rc4[:, c4, :]
                )

        def emit_outdma(i):
            dst = out[i * SB : (i + 1) * SB, :, :].rearrange(
                "(a pp) h d -> pp a h d", pp=TB
            )
            nc.gpsimd.dma_start(dst, out_all[:, i * CPG : (i + 1) * CPG, :, :])

        import functools

        for rep in range(repeat):
            units = [
                (p, i, j)
                for p in range(n_pairs)
                for i in range(NSB)
                for j in range(CPG * (i + 1))
            ]
            extras = {k: [] for k in range(len(units))}
            tail = []
            base_of = {}
            for k, (p, i, j) in enumerate(units):
                if j == 0:
                    base_of[(p, i)] = k

            def attach(p, i, tasks):
                nxt = (p, i + 1) if i + 1 < NSB else (p + 1, 0)
                if nxt not in base_of:
                    tail.extend(tasks)
                    return
                base = base_of[nxt]
                nu = CPG * (nxt[1] + 1)
                nt_ = len(tasks)
                # cap at unit nu-3: the next block's first qk is emitted (via
                # lookahead) during unit nu-1, and unit nu-2's extras follow
                # that block's own last-unit lookahead; staying two units
                # clear keeps every prep write emitted before its readers.
                cap = max(nu - 3, 0)
                for t_idx, task in enumerate(tasks):
                    k = base + 1 + min(t_idx * max(nu - 1, 1) // nt_, cap)
                    extras[k].append(task)

            for p in range(n_pairs):
                for i in range(NSB):
                    tasks = [
                        functools.partial(emit_norm, p, i, 0),
                        functools.partial(emit_norm, p, i, 1),
                    ]
                    if p == n_pairs - 1:
                        tasks.append(functools.partial(emit_outdma, i))
                    if p == 0:
                        # prep runs during s-block i+1; its chunks must be
                        # ready before s-block i+2's first qk is EMITTED, so
                        # prep chunks for block i+2 here (0..7 done upfront)
                        for c in range(CPG * (i + 2), min(CPG * (i + 3), NCH)):
                            tasks.append(functools.partial(prep_load, c))
                            tasks.append(functools.partial(prep_store, c))
                    attach(p, i, tasks)

            for c in range(min(2 * CPG, NCH)):
                prep_chunk(c)
            ps_cur = emit_qk(*units[0])
            for k, u in enumerate(units):
                pt = emit_exp(*u, ps_cur)
                ps_cur = emit_qk(*units[k + 1]) if k + 1 < len(units) else None
                emit_av(*u, pt)
                p_, i_, j_ = u
                if j_ == CPG * (i_ + 1) - 1:
                    emit_norm_copy(p_, i_, 0)
                    emit_norm_copy(p_, i_, 1)
                for task in extras[k]:
                    task()
            for task in tail:
                task()

    nc.compile()
    return nc


def get_nc(seq_len=S, n_heads=H * B // N_CORES, repeat=1):
    key = (seq_len, n_heads, repeat)
    if key not in _cache:
        _cache[key] = _build(seq_len, n_heads, repeat)
    return _cache[key]


def kernel(qkv: np.ndarray) -> np.ndarray:
    from concourse.bass_utils import run_bass_kernel_spmd

    qkv = np.ascontiguousarray(np.asarray(qkv, dtype=np.float32))
    assert qkv.shape == (B, S, 3, H, D)
    hpc = H * B // N_CORES  # heads per core
    cores_per_b = H // hpc
    ins = []
    for c in range(N_CORES):
        b, h0 = c // cores_per_b, (c % cores_per_b) * hpc
        ins.append({"qkv": np.ascontiguousarray(qkv[b, :, :, h0 : h0 + hpc, :])})
    nc = get_nc()
    res = run_bass_kernel_spmd(nc, ins, core_ids=list(range(N_CORES)))
    full = np.empty((B, S, H, D), np.float32)
    for c in range(N_CORES):
        b, h0 = c // cores_per_b, (c % cores_per_b) * hpc
        full[b, :, h0 : h0 + hpc, :] = res.results[c]["out"]
    return full
